# revision 31
# baseline (speedup 1.0000x reference)
"""MoE (top-2 of 64 experts, SwiGLU FFN) on 8 Trainium2 NeuronCores.

Strategy: expert-parallel. The router (softmax top-k over 64 experts) is
cheap bookkeeping and runs on the host in fp32; it determines how tokens
are dispatched, so it must run before device work can be sharded. Each
core owns 8 experts (weights sharded on the expert dim); tokens routed to
an expert are gathered on the host into that expert's panel, transposed
into PE-friendly layout and pre-cast to bf16. The device does all three
grouped GEMMs (h1 = x@w1.T, h3 = x@w3.T, out = (silu(h1)*h3)@w2.T) with
bf16 operands and fp32 PSUM accumulation. The host applies routing scores
and scatter-adds the two expert contributions per token (fp32).

Each expert is one "slot" processing a single token panel of S_j columns
(S_j static per slot, shared by all cores so the SPMD program is one
instruction stream). Panels are streamed through the PE in column groups
of <=512 so a weight chunk is loaded once per panel (LDWEIGHTS fully
hidden under the matmul stream). Experts are ranked by routed-token count
and rank r maps to slot r//8 on core r%8, so S_j (the max count in the
rank group) stays within ~5% of the per-core ideal.
"""

import os
import subprocess
import sys
import tempfile
import textwrap

import numpy as np
import ml_dtypes

import concourse.bass as bass
import concourse.mybir as mybir
import concourse.tile as tile
from concourse import bacc
from concourse.bass_utils import run_bass_kernel_spmd

# Model dims (hardcoded per contract; kernel.py must be self-contained).
T, DIM, HID, E = 16384, 1024, 512, 64
NCORES = 8
EPC = E // NCORES  # experts (slots) per core
KD = DIM // 128  # contraction chunks for GEMM1/3
MH = HID // 128  # output chunks for GEMM1/3
KH = HID // 128  # contraction chunks for GEMM2
MD = DIM // 128  # output chunks for GEMM2
ALIGN = 4  # panel size granularity
GMAX = 512  # max matmul free dim / PSUM bank columns (fp32)

f32 = mybir.dt.float32
bf16 = mybir.dt.bfloat16
AF = mybir.ActivationFunctionType
BF = ml_dtypes.bfloat16

_last_run = None  # BassKernelResults of the most recent run (for profiling)


def _route_numpy(x, gate_w, top_k):
    """Numpy router (fallback), same math as the reference in fp32."""
    logits = x @ gate_w.T
    m = logits.max(-1, keepdims=True)
    e = np.exp(logits - m)
    scores = e / e.sum(-1, keepdims=True)
    top_idx = np.argsort(-scores, axis=-1, kind="stable")[:, :top_k]
    top_sc = np.take_along_axis(scores, top_idx, axis=-1)
    return top_idx.astype(np.int64), top_sc.astype(np.float32)


def _route(x, gate_w, top_k):
    """Router on jax-CPU in a subprocess: bit-identical to the reference's
    softmax/top_k (near-tie top-k picks depend on matmul rounding, so exact
    replication avoids occasional expert flips). Falls back to numpy.
    """
    script = textwrap.dedent(
        """
        import numpy as np, sys
        import jax, jax.numpy as jnp
        assert jax.devices()[0].platform == "cpu"
        d = np.load(sys.argv[1] + "/in.npz")
        k = int(d["k"])
        scores = jax.nn.softmax(jnp.asarray(d["x"]) @ jnp.asarray(d["gw"]).T, axis=-1)
        ts, ti = jax.lax.top_k(scores, k)
        np.savez(sys.argv[1] + "/out.npz", ti=np.asarray(ti), ts=np.asarray(ts))
        """
    )
    try:
        with tempfile.TemporaryDirectory() as td:
            np.savez(td + "/in.npz", x=x, gw=gate_w, k=top_k)
            env = dict(os.environ)
            env["JAX_PLATFORMS"] = "cpu"
            env.pop("TRN_TERMINAL_POOL_IPS", None)  # skip the axon boot shim
            env["PYTHONPATH"] = os.pathsep.join(p for p in sys.path if p)
            r = subprocess.run(
                [sys.executable, "-c", script, td],
                env=env,
                capture_output=True,
                timeout=300,
            )
            if r.returncode != 0:
                raise RuntimeError(r.stderr.decode()[-500:])
            o = np.load(td + "/out.npz")
            return o["ti"].astype(np.int64), o["ts"].astype(np.float32)
    except Exception:
        return _route_numpy(x, gate_w, top_k)


def _groups(S):
    """Split a panel of S columns into >=2 groups of <=GMAX, ALIGN-rounded.

    Two (or more) groups let the silu/mult of group g overlap the matmuls
    of group g+1, so PSUM tiles recycle without stalling the PE.
    """
    n = max(2, -(-S // GMAX))
    per = -(-(S // n) // ALIGN) * ALIGN
    out = []
    c = 0
    while c < S:
        w = min(max(per, ALIGN), S - c, GMAX)
        out.append((c, w))
        c += w
    return out


def _build_program(S):
    """One SPMD program for all cores; S[j] = panel columns of slot j."""
    S = [int(s) for s in S]
    XTOT = sum(S)
    xoff = [0]
    for s in S:
        xoff.append(xoff[-1] + s)
    NG = max(len(_groups(s)) for s in S)
    po_bufs = max(2, 8 - 2 * NG)

    nc = bacc.Bacc()
    xin = nc.declare_dram_parameter("xin", [128, KD * XTOT], bf16, isOutput=False)
    w1t = nc.declare_dram_parameter("w1t", [EPC, 128, KD * HID], bf16, isOutput=False)
    w3t = nc.declare_dram_parameter("w3t", [EPC, 128, KD * HID], bf16, isOutput=False)
    w2t = nc.declare_dram_parameter("w2t", [EPC, 128, KH * DIM], bf16, isOutput=False)
    outd = nc.declare_dram_parameter("outd", [128, MD * XTOT], bf16, isOutput=True)

    with tile.TileContext(nc) as tc:
        with (
            tc.tile_pool(name="wp", bufs=2) as wp,
            tc.tile_pool(name="wp2", bufs=3) as wp2,
            tc.tile_pool(name="xp", bufs=2) as xp,
            tc.tile_pool(name="hp", bufs=2) as hp,
            tc.tile_pool(name="sp", bufs=4) as sp,
            tc.tile_pool(name="op", bufs=8) as op,
            tc.tile_pool(name="ph", bufs=1, space="PSUM") as ph,
            tc.tile_pool(name="po", bufs=po_bufs, space="PSUM") as po,
        ):
            SMAX = max(S)
            wsb = {}
            hts = {}

            def load_weights(j):
                w1sb = wp.tile([128, KD * HID], bf16, tag="w1")
                w3sb = wp.tile([128, KD * HID], bf16, tag="w3")
                w2sb = wp2.tile([128, KH * DIM], bf16, tag="w2")
                half = (KD // 2) * HID
                nc.sync.dma_start(w1sb[:, :half], w1t[j][:, :half])
                nc.sync.dma_start(w3sb[:, :half], w3t[j][:, :half])
                nc.sync.dma_start(w1sb[:, half:], w1t[j][:, half:])
                nc.sync.dma_start(w3sb[:, half:], w3t[j][:, half:])
                nc.sync.dma_start(w2sb[:], w2t[j][:])
                wsb[j] = (w1sb, w3sb, w2sb)

            xts = {}

            def load_x(j):
                Sj = S[j]
                xt = xp.tile([128, KD * SMAX], bf16, tag="xt", name="xt")[:, : KD * Sj]
                half = (KD // 2) * Sj
                src = xin[:, KD * xoff[j] : KD * xoff[j] + KD * Sj]
                nc.sync.dma_start(xt[:, :half], src[:, :half])
                nc.sync.dma_start(xt[:, half:], src[:, half:])
                xts[j] = xt

            def load_first(j):
                # startup: interleave x / w1 / w3 quarters so the first
                # matmuls start as soon as the first k-chunks land
                Sj = S[j]
                xt = xp.tile([128, KD * SMAX], bf16, tag="xt", name="xt")[:, : KD * Sj]
                src = xin[:, KD * xoff[j] : KD * xoff[j] + KD * Sj]
                w1sb = wp.tile([128, KD * HID], bf16, tag="w1", name="w1sb")
                w3sb = wp.tile([128, KD * HID], bf16, tag="w3", name="w3sb")
                w2sb = wp2.tile([128, KH * DIM], bf16, tag="w2", name="w2sb")
                xq = (KD // 4) * Sj
                wq = (KD // 4) * HID
                for q in range(4):
                    nc.sync.dma_start(
                        w1sb[:, q * wq : (q + 1) * wq], w1t[j][:, q * wq : (q + 1) * wq]
                    )
                    nc.sync.dma_start(
                        xt[:, q * xq : (q + 1) * xq], src[:, q * xq : (q + 1) * xq]
                    )
                    nc.sync.dma_start(
                        w3sb[:, q * wq : (q + 1) * wq], w3t[j][:, q * wq : (q + 1) * wq]
                    )
                nc.sync.dma_start(w2sb[:], w2t[j][:])
                wsb[j] = (w1sb, w3sb, w2sb)
                xts[j] = xt

            def gemm1(j):
                """x panel -> h panel (bf16, [128, KH*S]) for slot j."""
                w1sb, w3sb, _ = wsb[j]
                Sj = S[j]
                grps = _groups(Sj)
                xt = xts.pop(j)
                h = hp.tile([128, KH * SMAX], bf16, tag="h", name="h")[:, : KH * Sj]
                for m in range(MH):
                    for gi, (c0, cn) in enumerate(grps):
                        p1 = ph.tile([128, GMAX], f32, tag=f"p1_{gi}", name=f"p1_{gi}")[:, :cn]
                        p3 = ph.tile([128, GMAX], f32, tag=f"p3_{gi}", name=f"p3_{gi}")[:, :cn]
                        for k in range(KD):
                            nc.tensor.matmul(
                                p1[:],
                                w1sb[:, k * HID + m * 128 : k * HID + (m + 1) * 128],
                                xt[:, k * Sj + c0 : k * Sj + c0 + cn],
                                start=(k == 0),
                                stop=(k == KD - 1),
                            )
                        for k in range(KD):
                            nc.tensor.matmul(
                                p3[:],
                                w3sb[:, k * HID + m * 128 : k * HID + (m + 1) * 128],
                                xt[:, k * Sj + c0 : k * Sj + c0 + cn],
                                start=(k == 0),
                                stop=(k == KD - 1),
                            )
                        s = sp.tile([128, GMAX], f32, tag="s", name="s")[:, :cn]
                        nc.scalar.activation(s[:], p1[:], AF.Silu)
                        nc.vector.tensor_mul(
                            h[:, m * Sj + c0 : m * Sj + c0 + cn], s[:], p3[:]
                        )
                hts[j] = h

            def gemm2(j):
                _, _, w2sb = wsb[j]
                Sj = S[j]
                grps = _groups(Sj)
                h = hts.pop(j)
                for m2 in range(MD):
                    m2o = m2 % 2
                    if m2o == 0:
                        ot2 = op.tile([128, 2 * SMAX], bf16, tag="ot", name="ot")[
                            :, : 2 * Sj
                        ]
                    for gi, (c0, cn) in enumerate(grps):
                        pod = po.tile([128, GMAX], f32, tag="po", name="po")[:, :cn]
                        for k2 in range(KH):
                            nc.tensor.matmul(
                                pod[:],
                                w2sb[:, k2 * DIM + m2 * 128 : k2 * DIM + (m2 + 1) * 128],
                                h[:, k2 * Sj + c0 : k2 * Sj + c0 + cn],
                                start=(k2 == 0),
                                stop=(k2 == KH - 1),
                            )
                        if (m2 + gi) % 2 == 0:
                            nc.vector.tensor_copy(ot2[:, m2o * Sj + c0 :][:, :cn], pod[:])
                        else:
                            nc.scalar.activation(
                                ot2[:, m2o * Sj + c0 :][:, :cn], pod[:], AF.Copy
                            )
                    if m2o == 1:
                        nc.sync.dma_start(
                            outd[
                                :,
                                MD * xoff[j] + (m2 - 1) * Sj : MD * xoff[j]
                                + (m2 + 1) * Sj,
                            ],
                            ot2[:],
                        )

            # PE warm-up: the HAM clock gate needs ~3.4us of activity to
            # lift the PE from 1.2 to 2.4 GHz; burn the initial DMA wait on
            # dummy matmuls so the first real matmuls run at full clock.
            scr = sp.tile([128, GMAX], f32, tag="s", name="scr")
            nc.gpsimd.memset(scr[:], 0.0)
            for _ in range(10):
                pw = po.tile([128, GMAX], f32, tag="po", name="pw")
                nc.tensor.matmul(
                    pw[:],
                    scr[:, :128].bitcast(bf16)[:, :128],
                    scr[:].bitcast(bf16)[:, :GMAX],
                    start=True,
                    stop=True,
                )
            load_first(0)
            if EPC > 1:
                load_x(1)
                load_weights(1)
            for j in range(EPC):
                gemm1(j)
                if j + 2 < EPC:
                    load_x(j + 2)
                    load_weights(j + 2)
                if j > 0:
                    gemm2(j - 1)
            gemm2(EPC - 1)
    nc.compile()
    return nc


def kernel(x, gate_w, w1, w2, w3, top_k):
    x = np.asarray(x, dtype=np.float32)
    gate_w = np.asarray(gate_w, dtype=np.float32)
    w1 = np.asarray(w1, dtype=np.float32)
    w2 = np.asarray(w2, dtype=np.float32)
    w3 = np.asarray(w3, dtype=np.float32)
    k = int(top_k)

    top_idx, top_sc = _route(x, gate_w, k)

    # token lists per expert (dispatch order = row-major scan of (t, slot))
    tok_of = [np.nonzero(top_idx == e)[0] for e in range(E)]
    counts = np.array([len(t) for t in tok_of])

    # rank experts by count desc; rank r -> (core r % 8, slot r // 8);
    # slot panel size = max count in the rank group, ALIGN-rounded
    order = np.argsort(-counts, kind="stable")
    assign = [[0] * EPC for _ in range(NCORES)]
    for r, e in enumerate(order):
        assign[r % NCORES][r // NCORES] = int(e)
    S = [
        max(ALIGN, -(-int(counts[order[j * NCORES]]) // ALIGN) * ALIGN)
        for j in range(EPC)
    ]
    XTOT = sum(S)
    xoff = np.concatenate([[0], np.cumsum(S)])

    nc = _build_program(S)

    in_maps = []
    for c in range(NCORES):
        xin = np.zeros((128, KD * XTOT), dtype=BF)
        w1t = np.empty((EPC, 128, KD * HID), dtype=BF)
        w3t = np.empty((EPC, 128, KD * HID), dtype=BF)
        w2t = np.empty((EPC, 128, KH * DIM), dtype=BF)
        for j in range(EPC):
            e = assign[c][j]
            toks = tok_of[e]
            n = len(toks)
            assert n <= S[j]
            # x rows [n, DIM] -> [128(p), KD(k), n(col)]: d = k*128+p
            blk = np.zeros((128, KD, S[j]), dtype=BF)
            blk[:, :, :n] = x[toks].reshape(n, KD, 128).transpose(2, 1, 0).astype(BF)
            xin[:, KD * xoff[j] : KD * xoff[j + 1]] = blk.reshape(128, -1)
            w1t[j] = (
                w1[e].T.reshape(KD, 128, HID).transpose(1, 0, 2).reshape(128, -1)
            ).astype(BF)
            w3t[j] = (
                w3[e].T.reshape(KD, 128, HID).transpose(1, 0, 2).reshape(128, -1)
            ).astype(BF)
            w2t[j] = (
                w2[e].T.reshape(KH, 128, DIM).transpose(1, 0, 2).reshape(128, -1)
            ).astype(BF)
        in_maps.append({"xin": xin, "w1t": w1t, "w3t": w3t, "w2t": w2t})

    res = run_bass_kernel_spmd(nc, in_maps, list(range(NCORES)))
    global _last_run
    _last_run = res

    # out rows per core: [128(p), MD*XTOT] slot-major -> [XTOT, DIM]: d = m*128+p
    rows = np.empty((NCORES, XTOT, DIM), dtype=np.float32)
    for c in range(NCORES):
        o = res.results[c]["outd"].astype(np.float32)  # [128, MD*XTOT]
        for j in range(EPC):
            blk = o[:, MD * xoff[j] : MD * xoff[j + 1]].reshape(128, MD, S[j])
            rows[c, xoff[j] : xoff[j + 1]] = blk.transpose(2, 1, 0).reshape(S[j], DIM)

    # combine: for each (token, slot) pair find its dispatch position
    pos_core = np.empty((T, k), dtype=np.int64)
    pos_row = np.empty((T, k), dtype=np.int64)
    core_of = np.empty(E, dtype=np.int64)
    slot_of = np.empty(E, dtype=np.int64)
    for c in range(NCORES):
        for j in range(EPC):
            core_of[assign[c][j]] = c
            slot_of[assign[c][j]] = j
    for e in range(E):
        toks = tok_of[e]
        sl = np.nonzero(top_idx[toks] == e)[1]  # which top-k column chose e
        pos_core[toks, sl] = core_of[e]
        pos_row[toks, sl] = xoff[slot_of[e]] + np.arange(len(toks))
    y = np.zeros((T, DIM), dtype=np.float32)
    for s in range(k):
        y += top_sc[:, s : s + 1] * rows[pos_core[:, s], pos_row[:, s]]
    return y


# revision 32
# speedup vs baseline: 1.0258x; 1.0258x over previous
"""MoE (top-2 of 64 experts, SwiGLU FFN) on 8 Trainium2 NeuronCores.

Strategy: expert-parallel. The router (softmax top-k over 64 experts) is
cheap bookkeeping and runs on the host in fp32; it determines how tokens
are dispatched, so it must run before device work can be sharded. Each
core owns 8 experts (weights sharded on the expert dim); tokens routed to
an expert are gathered on the host into that expert's panel, transposed
into PE-friendly layout and pre-cast to bf16. The device does all three
grouped GEMMs (h1 = x@w1.T, h3 = x@w3.T, out = (silu(h1)*h3)@w2.T) with
bf16 operands and fp32 PSUM accumulation. The host applies routing scores
and scatter-adds the two expert contributions per token (fp32).

Each expert is one "slot" processing a single token panel of S_j columns
(S_j static per slot, shared by all cores so the SPMD program is one
instruction stream). Panels are streamed through the PE in column groups
of <=512 so a weight chunk is loaded once per panel (LDWEIGHTS fully
hidden under the matmul stream). Experts are ranked by routed-token count
and rank r maps to slot r//8 on core r%8, so S_j (the max count in the
rank group) stays within ~5% of the per-core ideal.
"""

import os
import subprocess
import sys
import tempfile
import textwrap

import numpy as np
import ml_dtypes

import concourse.bass as bass
import concourse.mybir as mybir
import concourse.tile as tile
from concourse import bacc
from concourse.bass_utils import run_bass_kernel_spmd

# Model dims (hardcoded per contract; kernel.py must be self-contained).
T, DIM, HID, E = 16384, 1024, 512, 64
NCORES = 8
EPC = E // NCORES  # experts (slots) per core
KD = DIM // 128  # contraction chunks for GEMM1/3
MH = HID // 128  # output chunks for GEMM1/3
KH = HID // 128  # contraction chunks for GEMM2
MD = DIM // 128  # output chunks for GEMM2
ALIGN = 4  # panel size granularity
GMAX = 512  # max matmul free dim / PSUM bank columns (fp32)

f32 = mybir.dt.float32
bf16 = mybir.dt.bfloat16
AF = mybir.ActivationFunctionType
BF = ml_dtypes.bfloat16

_last_run = None  # BassKernelResults of the most recent run (for profiling)


def _route_numpy(x, gate_w, top_k):
    """Numpy router (fallback), same math as the reference in fp32."""
    logits = x @ gate_w.T
    m = logits.max(-1, keepdims=True)
    e = np.exp(logits - m)
    scores = e / e.sum(-1, keepdims=True)
    top_idx = np.argsort(-scores, axis=-1, kind="stable")[:, :top_k]
    top_sc = np.take_along_axis(scores, top_idx, axis=-1)
    return top_idx.astype(np.int64), top_sc.astype(np.float32)


def _route(x, gate_w, top_k):
    """Router on jax-CPU in a subprocess: bit-identical to the reference's
    softmax/top_k (near-tie top-k picks depend on matmul rounding, so exact
    replication avoids occasional expert flips). Falls back to numpy.
    """
    script = textwrap.dedent(
        """
        import numpy as np, sys
        import jax, jax.numpy as jnp
        assert jax.devices()[0].platform == "cpu"
        d = np.load(sys.argv[1] + "/in.npz")
        k = int(d["k"])
        scores = jax.nn.softmax(jnp.asarray(d["x"]) @ jnp.asarray(d["gw"]).T, axis=-1)
        ts, ti = jax.lax.top_k(scores, k)
        np.savez(sys.argv[1] + "/out.npz", ti=np.asarray(ti), ts=np.asarray(ts))
        """
    )
    try:
        with tempfile.TemporaryDirectory() as td:
            np.savez(td + "/in.npz", x=x, gw=gate_w, k=top_k)
            env = dict(os.environ)
            env["JAX_PLATFORMS"] = "cpu"
            env.pop("TRN_TERMINAL_POOL_IPS", None)  # skip the axon boot shim
            env["PYTHONPATH"] = os.pathsep.join(p for p in sys.path if p)
            r = subprocess.run(
                [sys.executable, "-c", script, td],
                env=env,
                capture_output=True,
                timeout=300,
            )
            if r.returncode != 0:
                raise RuntimeError(r.stderr.decode()[-500:])
            o = np.load(td + "/out.npz")
            return o["ti"].astype(np.int64), o["ts"].astype(np.float32)
    except Exception:
        return _route_numpy(x, gate_w, top_k)


def _groups(S):
    """Split a panel of S columns into >=2 groups of <=GMAX, ALIGN-rounded.

    Two (or more) groups let the silu/mult of group g overlap the matmuls
    of group g+1, so PSUM tiles recycle without stalling the PE.
    """
    n = max(2, -(-S // GMAX))
    per = -(-(S // n) // ALIGN) * ALIGN
    out = []
    c = 0
    while c < S:
        w = min(max(per, ALIGN), S - c, GMAX)
        out.append((c, w))
        c += w
    return out


def _build_program(S):
    """One SPMD program for all cores; S[j] = panel columns of slot j."""
    S = [int(s) for s in S]
    XTOT = sum(S)
    xoff = [0]
    for s in S:
        xoff.append(xoff[-1] + s)
    NG = max(len(_groups(s)) for s in S)
    po_bufs = max(2, 8 - 2 * NG)

    nc = bacc.Bacc()
    xin = nc.declare_dram_parameter("xin", [128, KD * XTOT], bf16, isOutput=False)
    w1t = nc.declare_dram_parameter("w1t", [EPC, 128, KD * HID], bf16, isOutput=False)
    w3t = nc.declare_dram_parameter("w3t", [EPC, 128, KD * HID], bf16, isOutput=False)
    w2t = nc.declare_dram_parameter("w2t", [EPC, 128, KH * DIM], bf16, isOutput=False)
    outd = nc.declare_dram_parameter("outd", [128, MD * XTOT], bf16, isOutput=True)

    with tile.TileContext(nc) as tc:
        with (
            tc.tile_pool(name="wp", bufs=2) as wp,
            tc.tile_pool(name="wp2", bufs=3) as wp2,
            tc.tile_pool(name="xp", bufs=2) as xp,
            tc.tile_pool(name="hp", bufs=2) as hp,
            tc.tile_pool(name="sp", bufs=4) as sp,
            tc.tile_pool(name="op", bufs=8) as op,
            tc.tile_pool(name="ph", bufs=1, space="PSUM") as ph,
            tc.tile_pool(name="po", bufs=po_bufs, space="PSUM") as po,
        ):
            SMAX = max(S)
            wsb = {}
            hts = {}

            def load_weights(j):
                w1sb = wp.tile([128, KD * HID], bf16, tag="w1")
                w3sb = wp.tile([128, KD * HID], bf16, tag="w3")
                w2sb = wp2.tile([128, KH * DIM], bf16, tag="w2")
                half = (KD // 2) * HID
                nc.sync.dma_start(w1sb[:, :half], w1t[j][:, :half])
                nc.sync.dma_start(w3sb[:, :half], w3t[j][:, :half])
                nc.sync.dma_start(w1sb[:, half:], w1t[j][:, half:])
                nc.sync.dma_start(w3sb[:, half:], w3t[j][:, half:])
                nc.sync.dma_start(w2sb[:], w2t[j][:])
                wsb[j] = (w1sb, w3sb, w2sb)

            xts = {}

            def load_x(j):
                Sj = S[j]
                xt = xp.tile([128, KD * SMAX], bf16, tag="xt", name="xt")[:, : KD * Sj]
                half = (KD // 2) * Sj
                src = xin[:, KD * xoff[j] : KD * xoff[j] + KD * Sj]
                nc.sync.dma_start(xt[:, :half], src[:, :half])
                nc.sync.dma_start(xt[:, half:], src[:, half:])
                xts[j] = xt

            def load_first(j):
                # startup: interleave x / w1 / w3 quarters so the first
                # matmuls start as soon as the first k-chunks land
                Sj = S[j]
                xt = xp.tile([128, KD * SMAX], bf16, tag="xt", name="xt")[:, : KD * Sj]
                src = xin[:, KD * xoff[j] : KD * xoff[j] + KD * Sj]
                w1sb = wp.tile([128, KD * HID], bf16, tag="w1", name="w1sb")
                w3sb = wp.tile([128, KD * HID], bf16, tag="w3", name="w3sb")
                w2sb = wp2.tile([128, KH * DIM], bf16, tag="w2", name="w2sb")
                xq = (KD // 4) * Sj
                wq = (KD // 4) * HID
                for q in range(4):
                    nc.sync.dma_start(
                        w1sb[:, q * wq : (q + 1) * wq], w1t[j][:, q * wq : (q + 1) * wq]
                    )
                    nc.sync.dma_start(
                        xt[:, q * xq : (q + 1) * xq], src[:, q * xq : (q + 1) * xq]
                    )
                    nc.sync.dma_start(
                        w3sb[:, q * wq : (q + 1) * wq], w3t[j][:, q * wq : (q + 1) * wq]
                    )
                nc.sync.dma_start(w2sb[:], w2t[j][:])
                wsb[j] = (w1sb, w3sb, w2sb)
                xts[j] = xt

            def gemm1(j):
                """x panel -> h panel (bf16, [128, KH*S]) for slot j."""
                w1sb, w3sb, _ = wsb[j]
                Sj = S[j]
                grps = _groups(Sj)
                xt = xts.pop(j)
                h = hp.tile([128, KH * SMAX], bf16, tag="h", name="h")[:, : KH * Sj]
                for m in range(MH):
                    for gi, (c0, cn) in enumerate(grps):
                        p1 = ph.tile([128, GMAX], f32, tag=f"p1_{gi}", name=f"p1_{gi}")[:, :cn]
                        p3 = ph.tile([128, GMAX], f32, tag=f"p3_{gi}", name=f"p3_{gi}")[:, :cn]
                        for k in range(KD):
                            nc.tensor.matmul(
                                p1[:],
                                w1sb[:, k * HID + m * 128 : k * HID + (m + 1) * 128],
                                xt[:, k * Sj + c0 : k * Sj + c0 + cn],
                                start=(k == 0),
                                stop=(k == KD - 1),
                            )
                        for k in range(KD):
                            nc.tensor.matmul(
                                p3[:],
                                w3sb[:, k * HID + m * 128 : k * HID + (m + 1) * 128],
                                xt[:, k * Sj + c0 : k * Sj + c0 + cn],
                                start=(k == 0),
                                stop=(k == KD - 1),
                            )
                        s = sp.tile([128, GMAX], f32, tag="s", name="s")[:, :cn]
                        nc.scalar.activation(s[:], p1[:], AF.Silu)
                        nc.vector.tensor_mul(
                            h[:, m * Sj + c0 : m * Sj + c0 + cn], s[:], p3[:]
                        )
                hts[j] = h

            def gemm2(j):
                _, _, w2sb = wsb[j]
                Sj = S[j]
                grps = _groups(Sj)
                h = hts.pop(j)
                for m2 in range(MD):
                    m2o = m2 % 2
                    if m2o == 0:
                        ot2 = op.tile([128, 2 * SMAX], bf16, tag="ot", name="ot")[
                            :, : 2 * Sj
                        ]
                    for gi, (c0, cn) in enumerate(grps):
                        pod = po.tile([128, GMAX], f32, tag="po", name="po")[:, :cn]
                        for k2 in range(KH):
                            nc.tensor.matmul(
                                pod[:],
                                w2sb[:, k2 * DIM + m2 * 128 : k2 * DIM + (m2 + 1) * 128],
                                h[:, k2 * Sj + c0 : k2 * Sj + c0 + cn],
                                start=(k2 == 0),
                                stop=(k2 == KH - 1),
                            )
                        if (m2 + gi) % 2 == 0:
                            nc.vector.tensor_copy(ot2[:, m2o * Sj + c0 :][:, :cn], pod[:])
                        else:
                            nc.scalar.activation(
                                ot2[:, m2o * Sj + c0 :][:, :cn], pod[:], AF.Copy
                            )
                    if m2o == 1:
                        nc.sync.dma_start(
                            outd[
                                :,
                                MD * xoff[j] + (m2 - 1) * Sj : MD * xoff[j]
                                + (m2 + 1) * Sj,
                            ],
                            ot2[:],
                        )

            # PE warm-up: the HAM clock gate needs ~3.4us of activity to
            # lift the PE from 1.2 to 2.4 GHz; burn the initial DMA wait on
            # dummy matmuls so the first real matmuls run at full clock.
            scr = sp.tile([128, GMAX], f32, tag="s", name="scr")
            nc.gpsimd.memset(scr[:], 0.0)
            for _ in range(10):
                pw = po.tile([128, GMAX], f32, tag="po", name="pw")
                nc.tensor.matmul(
                    pw[:],
                    scr[:, :128].bitcast(bf16)[:, :128],
                    scr[:].bitcast(bf16)[:, :GMAX],
                    start=True,
                    stop=True,
                )
            load_first(0)
            if EPC > 1:
                load_x(1)
                load_weights(1)
            # a few more filler matmuls: the scheduler slots these into the
            # sub-us PE gaps while the DMA engines ramp up on slots 0-1,
            # keeping the HAM clock gate from re-throttling early on
            for _ in range(8):
                pw = po.tile([128, GMAX], f32, tag="po", name="pw")
                nc.tensor.matmul(
                    pw[:],
                    scr[:, :128].bitcast(bf16)[:, :128],
                    scr[:].bitcast(bf16)[:, :GMAX],
                    start=True,
                    stop=True,
                )
            for j in range(EPC):
                gemm1(j)
                if j + 2 < EPC:
                    load_x(j + 2)
                    load_weights(j + 2)
                if j > 0:
                    gemm2(j - 1)
            gemm2(EPC - 1)
    nc.compile()
    return nc


def kernel(x, gate_w, w1, w2, w3, top_k):
    x = np.asarray(x, dtype=np.float32)
    gate_w = np.asarray(gate_w, dtype=np.float32)
    w1 = np.asarray(w1, dtype=np.float32)
    w2 = np.asarray(w2, dtype=np.float32)
    w3 = np.asarray(w3, dtype=np.float32)
    k = int(top_k)

    top_idx, top_sc = _route(x, gate_w, k)

    # token lists per expert (dispatch order = row-major scan of (t, slot))
    tok_of = [np.nonzero(top_idx == e)[0] for e in range(E)]
    counts = np.array([len(t) for t in tok_of])

    # rank experts by count desc; rank r -> (core r % 8, slot r // 8);
    # slot panel size = max count in the rank group, ALIGN-rounded
    order = np.argsort(-counts, kind="stable")
    assign = [[0] * EPC for _ in range(NCORES)]
    for r, e in enumerate(order):
        assign[r % NCORES][r // NCORES] = int(e)
    S = [
        max(ALIGN, -(-int(counts[order[j * NCORES]]) // ALIGN) * ALIGN)
        for j in range(EPC)
    ]
    XTOT = sum(S)
    xoff = np.concatenate([[0], np.cumsum(S)])

    nc = _build_program(S)

    in_maps = []
    for c in range(NCORES):
        xin = np.zeros((128, KD * XTOT), dtype=BF)
        w1t = np.empty((EPC, 128, KD * HID), dtype=BF)
        w3t = np.empty((EPC, 128, KD * HID), dtype=BF)
        w2t = np.empty((EPC, 128, KH * DIM), dtype=BF)
        for j in range(EPC):
            e = assign[c][j]
            toks = tok_of[e]
            n = len(toks)
            assert n <= S[j]
            # x rows [n, DIM] -> [128(p), KD(k), n(col)]: d = k*128+p
            blk = np.zeros((128, KD, S[j]), dtype=BF)
            blk[:, :, :n] = x[toks].reshape(n, KD, 128).transpose(2, 1, 0).astype(BF)
            xin[:, KD * xoff[j] : KD * xoff[j + 1]] = blk.reshape(128, -1)
            w1t[j] = (
                w1[e].T.reshape(KD, 128, HID).transpose(1, 0, 2).reshape(128, -1)
            ).astype(BF)
            w3t[j] = (
                w3[e].T.reshape(KD, 128, HID).transpose(1, 0, 2).reshape(128, -1)
            ).astype(BF)
            w2t[j] = (
                w2[e].T.reshape(KH, 128, DIM).transpose(1, 0, 2).reshape(128, -1)
            ).astype(BF)
        in_maps.append({"xin": xin, "w1t": w1t, "w3t": w3t, "w2t": w2t})

    res = run_bass_kernel_spmd(nc, in_maps, list(range(NCORES)))
    global _last_run
    _last_run = res

    # out rows per core: [128(p), MD*XTOT] slot-major -> [XTOT, DIM]: d = m*128+p
    rows = np.empty((NCORES, XTOT, DIM), dtype=np.float32)
    for c in range(NCORES):
        o = res.results[c]["outd"].astype(np.float32)  # [128, MD*XTOT]
        for j in range(EPC):
            blk = o[:, MD * xoff[j] : MD * xoff[j + 1]].reshape(128, MD, S[j])
            rows[c, xoff[j] : xoff[j + 1]] = blk.transpose(2, 1, 0).reshape(S[j], DIM)

    # combine: for each (token, slot) pair find its dispatch position
    pos_core = np.empty((T, k), dtype=np.int64)
    pos_row = np.empty((T, k), dtype=np.int64)
    core_of = np.empty(E, dtype=np.int64)
    slot_of = np.empty(E, dtype=np.int64)
    for c in range(NCORES):
        for j in range(EPC):
            core_of[assign[c][j]] = c
            slot_of[assign[c][j]] = j
    for e in range(E):
        toks = tok_of[e]
        sl = np.nonzero(top_idx[toks] == e)[1]  # which top-k column chose e
        pos_core[toks, sl] = core_of[e]
        pos_row[toks, sl] = xoff[slot_of[e]] + np.arange(len(toks))
    y = np.zeros((T, DIM), dtype=np.float32)
    for s in range(k):
        y += top_sc[:, s : s + 1] * rows[pos_core[:, s], pos_row[:, s]]
    return y
